# revision 11
# baseline (speedup 1.0000x reference)
"""Trainium2 kernel for nn_DenseGeneral fp8-qdq forward.

Reference computes: out = qdq_e4m3(inputs) @ qdq_e4m3(kernel) + bf16_round(bias)
(forward pass of fp8-aware DenseGeneral; scale/amax updates only live in the
custom_vjp residuals and do not affect the forward output).

Strategy:
- Host: quantize inputs/kernel to e4m3 exactly as the reference does (scales are
  ones in this problem, but general scales are folded back into the output).
  OCP e4m3fn bit patterns == TRN FP8_EXP4 for |v| <= 240, which holds here.
- Shard 8-way over rows of inp_mat (data parallel); the kernel matrix is
  replicated (16 MB fp8 fits in SBUF alongside the 2 MB activation slice).
- Device: custom Tile kernel — all inputs SBUF-resident, 1024 fp8 DoubleRow
  matmuls per core issued at the 216 ns streaming floor, fp32 PSUM, c-major
  order so compute streams behind the FIFO input DMAs with no gaps; dummy-MM
  warmup opens the PE HAM clock gate during the initial DMA fill.
- Host: gather, apply scale product + bias, reshape.
"""

import numpy as np
import ml_dtypes

P = 128
B, S, D, F = 4, 2048, 2048, 8192
M = B * S  # 8192 rows of inp_mat
GRID_M, GRID_N = 8, 1
M_LOC = M // GRID_M  # 2048
N_LOC = F // GRID_N  # 4096
N_CORES = 8

_PROGRAM = None  # (nc, kxm_name, kxn_name, mxn_name)


def _build_program():
    """Custom resident-input fp8 DoubleRow matmul: per core
    out[M_LOC, N_LOC] = kxm.T @ kxn, all inputs SBUF-resident (12 MB),
    1024 DoubleRow MMs, c-major streaming so the PE runs gapless once the
    first n-column lands. Inputs on sync HWDGE (FIFO arrival), outputs on
    scalar HWDGE, copybacks on vector; short dummy-MM warmup opens the
    PE HAM clock gate before real work."""
    global _PROGRAM
    if _PROGRAM is not None:
        return _PROGRAM
    import concourse.bacc as bacc
    import concourse.mybir as mybir
    import concourse.tile as tile

    KO = D // P  # 16 k subtiles
    KP = KO // 2  # 8 k pairs
    MT = M_LOC // P  # 16 m tiles
    NCH = N_LOC // 512  # 8 n chunks
    N_WARM = 48

    nc = bacc.Bacc("TRN2", target_bir_lowering=False, debug=False)
    dt = mybir.dt
    DR = mybir.MatmulPerfMode.DoubleRow

    with tile.TileContext(nc) as tc:
        kxm = nc.dram_tensor(
            "kxm", (P, KO, M_LOC), dt.float8e4, kind="ExternalInput"
        ).ap()
        kxn = nc.dram_tensor(
            "kxn", (P, KO, N_LOC), dt.float8e4, kind="ExternalInput"
        ).ap()
        mxn = nc.dram_tensor(
            "mxn", (P, MT, N_LOC), dt.float32, kind="ExternalOutput"
        ).ap()

        with (
            tc.tile_pool(name="resident", bufs=1) as res_pool,
            tc.tile_pool(name="psum", bufs=7, space="PSUM") as psum_pool,
            tc.tile_pool(name="warmpsum", bufs=1, space="PSUM") as warm_psum,
            tc.tile_pool(name="outp", bufs=6) as out_pool,
            tc.tile_pool(name="warm", bufs=1) as warm_pool,
        ):
            # PE warmup: dummy matmuls on const tiles, no input deps.
            wt = warm_pool.tile([P, P], dt.float8e4, name="warm_w")
            mv = warm_pool.tile([P, 256], dt.float8e4, name="warm_m")
            nc.gpsimd.memset(wt[:], 0.0)
            nc.gpsimd.memset(mv[:], 0.0)
            wps = warm_psum.tile([P, 256], dt.float32, name="warm_ps")
            for _ in range(N_WARM):
                nc.tensor.matmul(wps[:], wt[:], mv[:], start=True, stop=True)
            wsb = warm_pool.tile([P, 1], dt.float32, name="warm_sink")
            nc.vector.tensor_copy(wsb[:], wps[:, 0:1])

            # Merged resident tiles: per-partition byte layout matches the
            # DRAM images, so one DMA carries all k-pairs for a column range.
            xw_all = res_pool.tile([P, KP, 2, M_LOC], dt.float8e4, name="xw_all")
            wn_all = res_pool.tile([P, KP, 2, N_LOC], dt.float8e4, name="wn_all")

            def ld_wn(jlo, jhi, c0, c1, eng=None):
                (eng or nc.sync).dma_start(
                    wn_all[:, jlo:jhi, :, c0 * 512 : c1 * 512],
                    kxn[:, 2 * jlo : 2 * jhi, c0 * 512 : c1 * 512],
                )

            def ld_xw(jlo, jhi, t0, t1, eng=None):
                (eng or nc.sync).dma_start(
                    xw_all[:, jlo:jhi, :, t0 * P : t1 * P],
                    kxm[:, 2 * jlo : 2 * jhi, t0 * P : t1 * P],
                )

            JH = KP // 2
            # Critical prefix on the otherwise-idle scalar HWDGE ring, in
            # parallel with the bulk loads on sync's ring (each FIFO).
            ld_wn(0, JH, 0, 1, eng=nc.scalar)
            ld_xw(0, KP, 0, 1, eng=nc.scalar)
            ld_wn(JH, KP, 0, 1, eng=nc.scalar)
            ld_xw(0, KP, 1, 4)
            ld_xw(0, KP, 4, MT)
            ld_wn(0, KP, 1, 2)
            ld_wn(0, KP, 2, 4)
            ld_wn(0, KP, 4, 8)
            ld_wn(0, KP, 8, NCH)

            # Main stream: c-major.
            for c in range(NCH):
                for t in range(MT):
                    ps = psum_pool.tile([P, 512], dt.float32, name="ps")
                    for j in range(KP):
                        nc.tensor.matmul(
                            ps[:],
                            xw_all[:, j, :, t * P : (t + 1) * P],
                            wn_all[:, j, :, c * 512 : (c + 1) * 512],
                            start=(j == 0),
                            stop=(j == KP - 1),
                            perf_mode=DR,
                        )
                    oc = out_pool.tile([P, 512], dt.float32, name="oc")
                    nc.vector.tensor_copy(oc[:], ps[:])
                    nc.scalar.dma_start(mxn[:, t, c * 512 : (c + 1) * 512], oc[:])

    nc.compile()
    _PROGRAM = (nc, "kxm", "kxn", "mxn")
    return _PROGRAM


def _qdq_e4m3(x, scale):
    """fp32 -> e4m3 with the reference's scale/clip semantics; TRN-fp8 view."""
    if scale != 1.0:
        x = x / np.float32(scale)
    q = np.clip(x, -448.0, 448.0).astype(ml_dtypes.float8_e4m3fn)
    return q.view(ml_dtypes.float8_e4m3)


def _to_partition_major(a):
    """[R, C] -> [P, R//P, C] with element [p, r, c] = a[r*P + p, c]."""
    R, C = a.shape
    return np.ascontiguousarray(a.reshape(R // P, P, C).transpose(1, 0, 2))


def _ensure_axon_hooks_stub():
    """The trimmed image lacks antenv.axon_hooks; if BASS_TRACE is set in the
    environment, run_bass_kernel_spmd would crash importing it. Provide a
    no-op hook module (bass_utils degrades gracefully on a None hook)."""
    import sys
    import types

    try:
        import antenv.axon_hooks  # noqa: F401
    except ImportError:
        mod = types.ModuleType("antenv.axon_hooks")
        mod.get_axon_ntff_profile_hook = lambda: None
        mod.set_axon_ntff_profile_hook = lambda hook: None
        sys.modules["antenv.axon_hooks"] = mod


def kernel(
    inputs,
    kernel,
    bias,
    input_scale,
    kernel_scale,
    output_grad_scale,
    input_amax_history,
    kernel_amax_history,
    output_grad_amax_history,
):
    _ensure_axon_hooks_stub()
    from concourse.bass_utils import run_bass_kernel_spmd

    nc, kxm_name, kxn_name, mxn_name = _build_program()

    x = np.asarray(inputs, dtype=np.float32).reshape(M, D)
    w = np.asarray(kernel, dtype=np.float32)
    s_in = float(np.asarray(input_scale).reshape(-1)[0])
    s_k = float(np.asarray(kernel_scale).reshape(-1)[0])

    xq = _qdq_e4m3(x, s_in)  # [M, D] fp8
    wq = _qdq_e4m3(w, s_k)  # [D, F] fp8

    # Per-shard DRAM images (shared across cores where slices coincide)
    kxm_shards = []
    for mi in range(GRID_M):
        xT = np.ascontiguousarray(xq[mi * M_LOC : (mi + 1) * M_LOC, :].T)  # [D, M_LOC]
        kxm_shards.append(_to_partition_major(xT))
    kxn_shards = []
    for ni in range(GRID_N):
        ws = wq[:, ni * N_LOC : (ni + 1) * N_LOC]  # [D, N_LOC]
        kxn_shards.append(_to_partition_major(np.ascontiguousarray(ws)))

    in_maps = []
    for c in range(N_CORES):
        mi, ni = divmod(c, GRID_N)
        in_maps.append({kxm_name: kxm_shards[mi], kxn_name: kxn_shards[ni]})

    res = run_bass_kernel_spmd(nc, in_maps, core_ids=list(range(N_CORES)))

    out = np.empty((M, F), dtype=np.float32)
    for c in range(N_CORES):
        mi, ni = divmod(c, GRID_N)
        block = res.results[c][mxn_name]  # [P, M_LOC//P, N_LOC]
        out[mi * M_LOC : (mi + 1) * M_LOC, ni * N_LOC : (ni + 1) * N_LOC] = (
            block.transpose(1, 0, 2).reshape(M_LOC, N_LOC)
        )

    sprod = s_in * s_k
    if sprod != 1.0:
        out *= np.float32(sprod)

    b = np.asarray(bias, dtype=np.float32)
    b = b.astype(ml_dtypes.bfloat16).astype(np.float32)
    if np.any(b):
        out += b[None, :]

    return out.reshape(B, S, F)


# revision 12
# speedup vs baseline: 1.0317x; 1.0317x over previous
"""Trainium2 kernel for nn_DenseGeneral fp8-qdq forward.

Reference computes: out = qdq_e4m3(inputs) @ qdq_e4m3(kernel) + bf16_round(bias)
(forward pass of fp8-aware DenseGeneral; scale/amax updates only live in the
custom_vjp residuals and do not affect the forward output).

Strategy:
- Host: quantize inputs/kernel to e4m3 exactly as the reference does (scales are
  ones in this problem, but general scales are folded back into the output).
  OCP e4m3fn bit patterns == TRN FP8_EXP4 for |v| <= 240, which holds here.
- Shard 8-way over rows of inp_mat (data parallel); the kernel matrix is
  replicated (16 MB fp8 fits in SBUF alongside the 2 MB activation slice).
- Device: custom Tile kernel — all inputs SBUF-resident, 1024 fp8 DoubleRow
  matmuls per core issued at the 216 ns streaming floor, fp32 PSUM, c-major
  order so compute streams behind the FIFO input DMAs with no gaps; dummy-MM
  warmup opens the PE HAM clock gate during the initial DMA fill.
- Host: gather, apply scale product + bias, reshape.
"""

import numpy as np
import ml_dtypes

P = 128
B, S, D, F = 4, 2048, 2048, 8192
M = B * S  # 8192 rows of inp_mat
GRID_M, GRID_N = 8, 1
M_LOC = M // GRID_M  # 2048
N_LOC = F // GRID_N  # 4096
N_CORES = 8

_PROGRAM = None  # (nc, kxm_name, kxn_name, mxn_name)


def _build_program():
    """Custom resident-input fp8 DoubleRow matmul: per core
    out[M_LOC, N_LOC] = kxm.T @ kxn, all inputs SBUF-resident (12 MB),
    1024 DoubleRow MMs, c-major streaming so the PE runs gapless once the
    first n-column lands. Inputs on sync HWDGE (FIFO arrival), outputs on
    scalar HWDGE, copybacks on vector; short dummy-MM warmup opens the
    PE HAM clock gate before real work."""
    global _PROGRAM
    if _PROGRAM is not None:
        return _PROGRAM
    import concourse.bacc as bacc
    import concourse.mybir as mybir
    import concourse.tile as tile

    KO = D // P  # 16 k subtiles
    KP = KO // 2  # 8 k pairs
    MT = M_LOC // P  # 16 m tiles
    NCH = N_LOC // 512  # 8 n chunks
    N_WARM = 48

    nc = bacc.Bacc("TRN2", target_bir_lowering=False, debug=False)
    dt = mybir.dt
    DR = mybir.MatmulPerfMode.DoubleRow

    with tile.TileContext(nc) as tc:
        kxm = nc.dram_tensor(
            "kxm", (P, KO, M_LOC), dt.float8e4, kind="ExternalInput"
        ).ap()
        kxn = nc.dram_tensor(
            "kxn", (P, KO, N_LOC), dt.float8e4, kind="ExternalInput"
        ).ap()
        mxn = nc.dram_tensor(
            "mxn", (P, MT, N_LOC), dt.float32, kind="ExternalOutput"
        ).ap()

        with (
            tc.tile_pool(name="resident", bufs=1) as res_pool,
            tc.tile_pool(name="psum", bufs=7, space="PSUM") as psum_pool,
            tc.tile_pool(name="warmpsum", bufs=1, space="PSUM") as warm_psum,
            tc.tile_pool(name="outp", bufs=6) as out_pool,
            tc.tile_pool(name="warm", bufs=1) as warm_pool,
        ):
            # PE warmup: dummy matmuls on const tiles, no input deps.
            wt = warm_pool.tile([P, P], dt.float8e4, name="warm_w")
            mv = warm_pool.tile([P, 256], dt.float8e4, name="warm_m")
            nc.gpsimd.memset(wt[:], 0.0)
            nc.gpsimd.memset(mv[:], 0.0)
            wps = warm_psum.tile([P, 256], dt.float32, name="warm_ps")
            for _ in range(N_WARM):
                nc.tensor.matmul(wps[:], wt[:], mv[:], start=True, stop=True)
            wsb = warm_pool.tile([P, 1], dt.float32, name="warm_sink")
            nc.vector.tensor_copy(wsb[:], wps[:, 0:1])

            # Merged resident tiles: per-partition byte layout matches the
            # DRAM images, so one DMA carries all k-pairs for a column range.
            xw_all = res_pool.tile([P, KP, 2, M_LOC], dt.float8e4, name="xw_all")
            wn_all = res_pool.tile([P, KP, 2, N_LOC], dt.float8e4, name="wn_all")

            def ld_wn(jlo, jhi, c0, c1, eng=None):
                (eng or nc.sync).dma_start(
                    wn_all[:, jlo:jhi, :, c0 * 512 : c1 * 512],
                    kxn[:, 2 * jlo : 2 * jhi, c0 * 512 : c1 * 512],
                )

            def ld_xw(jlo, jhi, t0, t1, eng=None):
                (eng or nc.sync).dma_start(
                    xw_all[:, jlo:jhi, :, t0 * P : t1 * P],
                    kxm[:, 2 * jlo : 2 * jhi, t0 * P : t1 * P],
                )

            JH = KP // 2
            # Critical prefix first on sync's FIFO ring: first n-column +
            # first m-tile, split for earlier partial arrival.
            ld_wn(0, JH, 0, 1)
            ld_xw(0, KP, 0, 1)
            ld_wn(JH, KP, 0, 1)
            ld_xw(0, KP, 1, 4)
            ld_xw(0, KP, 4, MT)
            ld_wn(0, KP, 1, 2)
            ld_wn(0, KP, 2, 4)
            ld_wn(0, KP, 4, 8)
            ld_wn(0, KP, 8, NCH)

            # Main stream: c-major.
            for c in range(NCH):
                for t in range(MT):
                    ps = psum_pool.tile([P, 512], dt.float32, name="ps")
                    for j in range(KP):
                        nc.tensor.matmul(
                            ps[:],
                            xw_all[:, j, :, t * P : (t + 1) * P],
                            wn_all[:, j, :, c * 512 : (c + 1) * 512],
                            start=(j == 0),
                            stop=(j == KP - 1),
                            perf_mode=DR,
                        )
                    oc = out_pool.tile([P, 512], dt.float32, name="oc")
                    nc.vector.tensor_copy(oc[:], ps[:])
                    nc.scalar.dma_start(mxn[:, t, c * 512 : (c + 1) * 512], oc[:])

    nc.compile()
    _PROGRAM = (nc, "kxm", "kxn", "mxn")
    return _PROGRAM


def _qdq_e4m3(x, scale):
    """fp32 -> e4m3 with the reference's scale/clip semantics; TRN-fp8 view."""
    if scale != 1.0:
        x = x / np.float32(scale)
    q = np.clip(x, -448.0, 448.0).astype(ml_dtypes.float8_e4m3fn)
    return q.view(ml_dtypes.float8_e4m3)


def _to_partition_major(a):
    """[R, C] -> [P, R//P, C] with element [p, r, c] = a[r*P + p, c]."""
    R, C = a.shape
    return np.ascontiguousarray(a.reshape(R // P, P, C).transpose(1, 0, 2))


def _ensure_axon_hooks_stub():
    """The trimmed image lacks antenv.axon_hooks; if BASS_TRACE is set in the
    environment, run_bass_kernel_spmd would crash importing it. Provide a
    no-op hook module (bass_utils degrades gracefully on a None hook)."""
    import sys
    import types

    try:
        import antenv.axon_hooks  # noqa: F401
    except ImportError:
        mod = types.ModuleType("antenv.axon_hooks")
        mod.get_axon_ntff_profile_hook = lambda: None
        mod.set_axon_ntff_profile_hook = lambda hook: None
        sys.modules["antenv.axon_hooks"] = mod


def kernel(
    inputs,
    kernel,
    bias,
    input_scale,
    kernel_scale,
    output_grad_scale,
    input_amax_history,
    kernel_amax_history,
    output_grad_amax_history,
):
    _ensure_axon_hooks_stub()
    from concourse.bass_utils import run_bass_kernel_spmd

    nc, kxm_name, kxn_name, mxn_name = _build_program()

    x = np.asarray(inputs, dtype=np.float32).reshape(M, D)
    w = np.asarray(kernel, dtype=np.float32)
    s_in = float(np.asarray(input_scale).reshape(-1)[0])
    s_k = float(np.asarray(kernel_scale).reshape(-1)[0])

    xq = _qdq_e4m3(x, s_in)  # [M, D] fp8
    wq = _qdq_e4m3(w, s_k)  # [D, F] fp8

    # Per-shard DRAM images (shared across cores where slices coincide)
    kxm_shards = []
    for mi in range(GRID_M):
        xT = np.ascontiguousarray(xq[mi * M_LOC : (mi + 1) * M_LOC, :].T)  # [D, M_LOC]
        kxm_shards.append(_to_partition_major(xT))
    kxn_shards = []
    for ni in range(GRID_N):
        ws = wq[:, ni * N_LOC : (ni + 1) * N_LOC]  # [D, N_LOC]
        kxn_shards.append(_to_partition_major(np.ascontiguousarray(ws)))

    in_maps = []
    for c in range(N_CORES):
        mi, ni = divmod(c, GRID_N)
        in_maps.append({kxm_name: kxm_shards[mi], kxn_name: kxn_shards[ni]})

    res = run_bass_kernel_spmd(nc, in_maps, core_ids=list(range(N_CORES)))

    out = np.empty((M, F), dtype=np.float32)
    for c in range(N_CORES):
        mi, ni = divmod(c, GRID_N)
        block = res.results[c][mxn_name]  # [P, M_LOC//P, N_LOC]
        out[mi * M_LOC : (mi + 1) * M_LOC, ni * N_LOC : (ni + 1) * N_LOC] = (
            block.transpose(1, 0, 2).reshape(M_LOC, N_LOC)
        )

    sprod = s_in * s_k
    if sprod != 1.0:
        out *= np.float32(sprod)

    b = np.asarray(bias, dtype=np.float32)
    b = b.astype(ml_dtypes.bfloat16).astype(np.float32)
    if np.any(b):
        out += b[None, :]

    return out.reshape(B, S, F)
